# revision 9
# baseline (speedup 1.0000x reference)
"""Trainium2 Bass kernel for nn_EdgeUpdater (GNN message passing edge update).

Computes, for E=2M edges with node tables of 100k rows (C=32):
    v = relu(relu(var_f @ Wv1.T + bv1) @ Wv2.T + bv2)
    c = relu(relu(con_f @ Wc1.T + bc1) @ Wc2.T + bc2)
    x = concat([combined_edge_f, v[i0], c[i1]], axis=1)      # [E, 3C]
    out = relu(x @ We1.T + be1) @ We2.T + be2                # [E, C]

Strategy (8 cores, edge-sharded, node tables replicated per core):
  Algebraic split of We1 = [A | B | Cc] over the concat:
    h  = relu(e @ A.T + Pv[i0] + Pc[i1]),  out = h @ We2.T + be2
    Pv = v @ B.T + be1   (be1 folded once),  Pc = c @ Cc.T
  Per core: compute Pv/Pc tables (fp16, DRAM), then stream edge shard:
  row-gather Pv[i0]+Pc[i1] via SWDGE indirect DMA (the second gather
  accumulates with accum_op=add), fp16 matmuls with 4x block-diagonal
  (kron(I4, W.T)) stationaries over a group-cyclic layout, DVE 32x32
  stream-transposes for row-major <-> channel-major conversion.
"""

import numpy as np

import concourse.bass as bass
import concourse.mybir as mybir
import concourse.tile as tile
from concourse.bass import IndirectOffsetOnAxis
from concourse.bass_utils import run_bass_kernel_spmd

C = 32
P = 128
NB = 64  # rows per partition per macro tile
MACRO = P * NB  # 8192 rows per macro tile
BANK = 512  # psum bank free dim (fp32)
NBANKS = (NB * C) // BANK  # 4
N_CORES = 8

F32 = mybir.dt.float32
F16 = mybir.dt.float16
I32 = mybir.dt.int32

RELU = mybir.ActivationFunctionType.Relu
IDENT = mybir.ActivationFunctionType.Identity
ADD = mybir.AluOpType.add


def _split_multi_waits(nc: bass.Bass, max_waits: int = 1):
    """The walrus in this container rejects instructions carrying more than
    one sync wait ("Too many sync wait commands", CoreV2/V3 setupSyncWait).
    Hoist extra waits onto single-wait NOPs on the same engine, inserted
    immediately before the instruction (same per-engine program order, so
    semantics are unchanged)."""
    for fn in nc.m.functions:
        for bb in fn.blocks:
            insts = list(bb.instructions)
            out = []
            for ins in insts:
                si = ins.sync_info
                if (si is not None and si.on_wait
                        and len(si.on_wait) > max_waits
                        and ins.engine is not None):
                    waits = list(si.on_wait)
                    eng = nc.engines[ins.engine]
                    for w in waits[:-max_waits]:
                        nop = eng.nop()
                        cur = nc.cur_bb.bb
                        assert cur.instructions[-1] is nop.ins
                        cur.instructions.pop()
                        nop.ins.sync_info = mybir.SyncInfo(
                            on_wait=[w], on_update=[])
                        out.append(nop.ins)
                    si.on_wait = waits[-max_waits:]
                out.append(ins)
            bb.instructions.clear()
            for ins in out:
                bb.instructions.append(ins)


def build_nc(me: int, mn: int) -> bass.Bass:
    """Build the per-core Bass module.

    me: number of 8192-edge macro tiles in the edge shard
    mn: number of 8192-row macro tiles per node table
    """
    nc = bass.Bass()

    ef = nc.declare_dram_parameter("ef", [me, P, NB, C], F32, isOutput=False)
    i0 = nc.declare_dram_parameter("i0", [me, P, NB], I32, isOutput=False)
    i1 = nc.declare_dram_parameter("i1", [me, P, NB], I32, isOutput=False)
    vf = nc.declare_dram_parameter("vf", [mn, P, NB, C], F32, isOutput=False)
    cf = nc.declare_dram_parameter("cf", [mn, P, NB, C], F32, isOutput=False)

    wnames = ["a_st", "w2_st", "ident", "wv1_st", "wv2_st", "pv_st",
              "wc1_st", "wc2_st", "pc_st"]
    wparams = {n: nc.declare_dram_parameter(n, [P, P], F16, isOutput=False)
               for n in wnames}
    bnames = ["bv1_t", "bv2_t", "bc1_t", "bc2_t", "be1_t", "be2_t"]
    bparams = {n: nc.declare_dram_parameter(n, [P, 1], F32, isOutput=False)
               for n in bnames}

    out = nc.declare_dram_parameter("out", [me, P, NB, C], F32, isOutput=True)

    n_nodes_pad = mn * MACRO
    pv_tab = nc.dram_tensor("pv_tab", [n_nodes_pad, C], F16)
    pc_tab = nc.dram_tensor("pc_tab", [n_nodes_pad, C], F16)

    with tile.TileContext(nc) as tc:
        with tc.tile_pool(name="const", bufs=1) as cpool:
            W = {}
            for n in wnames:
                t = cpool.tile([P, P], F16, tag=n)
                nc.sync.dma_start(t[:], wparams[n][:])
                W[n] = t
            B = {}
            for n in bnames:
                t = cpool.tile([P, 1], F32, tag=n)
                nc.sync.dma_start(t[:], bparams[n][:])
                B[n] = t

            # ---------------- node phase: build Pv / Pc tables ----------------
            with tc.tile_pool(name="nsb", bufs=2) as pool, \
                 tc.tile_pool(name="npsum", bufs=2, space="PSUM") as psum:

                def node_macro(src, tab_view, w1, w2, w3, b1, b2, b3, mi):
                    Lf = pool.tile([P, NB * C], F32, tag="nLf")
                    nc.sync.dma_start(Lf[:], src[mi])
                    L = pool.tile([P, NB * C], F16, tag="nL")
                    nc.vector.tensor_copy(L[:], Lf[:])  # f32 -> f16
                    X = pool.tile([P, NB * C], F16, tag="nX")
                    nc.vector.transpose(X[:], L[:])
                    pcm = pool.tile([P, NB * C], F16, tag="npcm")
                    for q in range(NBANKS):
                        sl = slice(q * BANK, (q + 1) * BANK)
                        ps1 = psum.tile([P, BANK], F32, tag="nps1")
                        nc.tensor.matmul(ps1[:], lhsT=w1[:], rhs=X[:, sl],
                                         start=True, stop=True)
                        l1 = pool.tile([P, BANK], F16, tag="nl1")
                        nc.scalar.activation(l1[:], ps1[:], RELU, bias=b1[:])
                        ps2 = psum.tile([P, BANK], F32, tag="nps2")
                        nc.tensor.matmul(ps2[:], lhsT=w2[:], rhs=l1[:],
                                         start=True, stop=True)
                        l2 = pool.tile([P, BANK], F16, tag="nl2")
                        nc.scalar.activation(l2[:], ps2[:], RELU, bias=b2[:])
                        ps3 = psum.tile([P, BANK], F32, tag="nps3")
                        nc.tensor.matmul(ps3[:], lhsT=w3[:], rhs=l2[:],
                                         start=True, stop=True)
                        if b3 is not None:
                            nc.scalar.activation(pcm[:, sl], ps3[:], IDENT,
                                                 bias=b3[:])
                        else:
                            nc.scalar.activation(pcm[:, sl], ps3[:], IDENT)
                    pem = pool.tile([P, NB * C], F16, tag="npem")
                    nc.vector.transpose(pem[:], pcm[:])
                    nc.sync.dma_start(tab_view[mi], pem[:])

                pv_view = pv_tab[:].rearrange("(m p b) c -> m p b c", p=P, b=NB)
                pc_view = pc_tab[:].rearrange("(m p b) c -> m p b c", p=P, b=NB)
                for mi in range(mn):
                    node_macro(vf, pv_view, W["wv1_st"], W["wv2_st"], W["pv_st"],
                               B["bv1_t"], B["bv2_t"], B["be1_t"], mi)
                for mi in range(mn):
                    node_macro(cf, pc_view, W["wc1_st"], W["wc2_st"], W["pc_st"],
                               B["bc1_t"], B["bc2_t"], None, mi)

            # ---------------- edge phase ----------------
            with tc.tile_pool(name="esb", bufs=3) as pool, \
                 tc.tile_pool(name="epsum", bufs=2, space="PSUM") as psum:
                for mi in range(me):
                    Lf = pool.tile([P, NB * C], F32, tag="eLf")
                    nc.sync.dma_start(Lf[:], ef[mi])
                    L = pool.tile([P, NB * C], F16, tag="eL")
                    nc.vector.tensor_copy(L[:], Lf[:])  # f32 -> f16
                    X = pool.tile([P, NB * C], F16, tag="eX")
                    nc.vector.transpose(X[:], L[:])

                    t0 = pool.tile([P, NB], I32, tag="ei0")
                    nc.sync.dma_start(t0[:], i0[mi])
                    t1 = pool.tile([P, NB], I32, tag="ei1")
                    nc.sync.dma_start(t1[:], i1[mi])

                    # HW indirect DMA gathers one row per partition per
                    # instruction (offset = first index of each partition row,
                    # span = out free extent), so gather row-column by
                    # row-column.
                    Sv = pool.tile([P, NB * C], F16, tag="eSv")
                    Sc = pool.tile([P, NB * C], F16, tag="eSc")
                    for s in range(NB):
                        nc.gpsimd.indirect_dma_start(
                            out=Sv[:, s * C:(s + 1) * C], out_offset=None,
                            in_=pv_tab[:],
                            in_offset=IndirectOffsetOnAxis(
                                ap=t0[:, s:s + 1], axis=0))
                    for s in range(NB):
                        nc.gpsimd.indirect_dma_start(
                            out=Sc[:, s * C:(s + 1) * C], out_offset=None,
                            in_=pc_tab[:],
                            in_offset=IndirectOffsetOnAxis(
                                ap=t1[:, s:s + 1], axis=0))
                    S = pool.tile([P, NB * C], F16, tag="eS")
                    nc.vector.tensor_add(S[:], Sv[:], Sc[:])
                    ST = pool.tile([P, NB * C], F16, tag="eST")
                    nc.vector.transpose(ST[:], S[:])

                    ocm = pool.tile([P, NB * C], F32, tag="eocm")
                    for q in range(NBANKS):
                        sl = slice(q * BANK, (q + 1) * BANK)
                        ps1 = psum.tile([P, BANK], F32, tag="eps1")
                        nc.tensor.matmul(ps1[:], lhsT=W["a_st"][:], rhs=X[:, sl],
                                         start=True, stop=False)
                        nc.tensor.matmul(ps1[:], lhsT=W["ident"][:], rhs=ST[:, sl],
                                         start=False, stop=True)
                        h = pool.tile([P, BANK], F16, tag="eh")
                        nc.scalar.activation(h[:], ps1[:], RELU)
                        ps2 = psum.tile([P, BANK], F32, tag="eps2")
                        nc.tensor.matmul(ps2[:], lhsT=W["w2_st"][:], rhs=h[:],
                                         start=True, stop=True)
                        nc.scalar.activation(ocm[:, sl], ps2[:], IDENT,
                                             bias=B["be2_t"][:])
                    oem = pool.tile([P, NB * C], F32, tag="eoem")
                    nc.vector.transpose(oem[:], ocm[:])
                    nc.sync.dma_start(out[mi], oem[:])

    _split_multi_waits(nc)
    return nc


def _kron4(w: np.ndarray) -> np.ndarray:
    return np.kron(np.eye(4, dtype=np.float32), w).astype(np.float16)


def _bias_t(b: np.ndarray) -> np.ndarray:
    return np.tile(np.asarray(b, np.float32), 4)[:, None].astype(np.float32)


def make_weight_inputs(Wv1, bv1, Wv2, bv2, Wc1, bc1, Wc2, bc2, We1, be1, We2, be2):
    We1 = np.asarray(We1, np.float32)
    return {
        "a_st": _kron4(np.asarray(We1[:, :C]).T.astype(np.float32)),
        "w2_st": _kron4(np.asarray(We2, np.float32).T),
        "ident": np.eye(P, dtype=np.float16),
        "wv1_st": _kron4(np.asarray(Wv1, np.float32).T),
        "wv2_st": _kron4(np.asarray(Wv2, np.float32).T),
        "pv_st": _kron4(We1[:, C:2 * C].T),
        "wc1_st": _kron4(np.asarray(Wc1, np.float32).T),
        "wc2_st": _kron4(np.asarray(Wc2, np.float32).T),
        "pc_st": _kron4(We1[:, 2 * C:3 * C].T),
        "bv1_t": _bias_t(bv1),
        "bv2_t": _bias_t(bv2),
        "bc1_t": _bias_t(bc1),
        "bc2_t": _bias_t(bc2),
        "be1_t": _bias_t(be1),
        "be2_t": _bias_t(be2),
    }


def _pad_nodes(x: np.ndarray, mn: int) -> np.ndarray:
    n_pad = mn * MACRO
    xp = np.zeros((n_pad, C), np.float32)
    xp[: x.shape[0]] = x
    return xp.reshape(mn, P, NB, C)


_NC_CACHE: dict = {}


def _get_nc(me: int, mn: int) -> bass.Bass:
    key = (me, mn)
    if key not in _NC_CACHE:
        _NC_CACHE[key] = build_nc(me, mn)
    return _NC_CACHE[key]


def kernel(var_f, con_f, combined_edge_f, edge_index_var_con,
           Wv1, bv1, Wv2, bv2, Wc1, bc1, Wc2, bc2, We1, be1, We2, be2,
           _trace=False, _tmpdir=None):
    var_f = np.asarray(var_f, np.float32)
    con_f = np.asarray(con_f, np.float32)
    combined_edge_f = np.asarray(combined_edge_f, np.float32)
    eidx = np.asarray(edge_index_var_con)

    E = combined_edge_f.shape[0]
    per = -(-E // N_CORES)
    me = -(-per // MACRO)
    e_pad = me * MACRO
    mn = -(-max(var_f.shape[0], con_f.shape[0]) // MACRO)

    base = make_weight_inputs(Wv1, bv1, Wv2, bv2, Wc1, bc1, Wc2, bc2,
                              We1, be1, We2, be2)
    base["vf"] = _pad_nodes(var_f, mn)
    base["cf"] = _pad_nodes(con_f, mn)

    i0_full = eidx[0].astype(np.int32)
    i1_full = eidx[1].astype(np.int32)

    in_maps = []
    shard_lens = []
    for k in range(N_CORES):
        lo = k * per
        hi = min(lo + per, E)
        n = hi - lo
        shard_lens.append(n)
        ef_k = np.zeros((e_pad, C), np.float32)
        ef_k[:n] = combined_edge_f[lo:hi]
        i0_k = np.zeros((e_pad,), np.int32)
        i0_k[:n] = i0_full[lo:hi]
        i1_k = np.zeros((e_pad,), np.int32)
        i1_k[:n] = i1_full[lo:hi]
        m = dict(base)
        m["ef"] = ef_k.reshape(me, P, NB, C)
        m["i0"] = i0_k.reshape(me, P, NB)
        m["i1"] = i1_k.reshape(me, P, NB)
        in_maps.append(m)

    nc = _get_nc(me, mn)
    res = run_bass_kernel_spmd(nc, in_maps, list(range(N_CORES)),
                               trace=_trace, tmpdir=_tmpdir)

    outs = []
    for k in range(N_CORES):
        o = np.asarray(res.results[k]["out"]).reshape(e_pad, C)
        outs.append(o[: shard_lens[k]])
    full = np.concatenate(outs, axis=0)
    if _trace:
        return full, res
    return full
